# revision 2
# baseline (speedup 1.0000x reference)
"""MixedScoreMultiHeadAttention on 8 TRN2 NeuronCores.

Sharding: data-parallel over batch B=8 (one batch element per core, no
collectives).  Per core (R=C=256, E=512, H=8, D=64, HID=128):

  1. QKV projections (bf16 matmuls; embeddings host-pretransposed to [E, S]).
  2. Per-head dot scores (K=64 matmuls, 2 heads packed via row groups).
  3. Channel-collapse via a DRAM bounce into S4 [32g+ch, pos] so the
     score-MLP runs channel-major with 4x tile_position row-packing (K=9).
  4. MLP waves (software-pipelined): W1 (4 concurrent row-tiled matmuls) ->
     per-group relu evict (ACT+DVE split, the elementwise bottleneck) ->
     W2 (4 concurrent col-tiled M=8 matmuls) -> DRAM-bounce scatter back to
     [r, (h, c)] logit tiles, half-rchunk granularity.
  5. Softmax without max-subtraction (logits are provably O(5)), mask applied
     multiplicatively after exp (fully-masked rows via +eps on the
     denominator), DMA-transpose of the weights, AV producing out^T per
     r-half, final projection per r-half -- all interleaved with the wave
     loop of the other row chunk.

The score-MLP weights are algebraically folded on the host:
  hidden = relu(concat_h[dot_h, alpha_h*cost] @ W1)
         = relu(sum_h dot_h * W1[2h,:] + cost * sum_h alpha_h W1[2h+1,:])
so the device sees a 9-channel input (8 raw-dot channels + 1 cost channel)
and an M9 [9, HID] matrix with the 1/sqrt(D) norm folded into the dot rows.
"""

import os

os.environ.setdefault("MYCRO_LOCAL_CACHE", "1")

import numpy as np
import ml_dtypes

import concourse.bass as bass
import concourse.mybir as mybir
import concourse.tile as tile
from concourse import bacc
from concourse.bass_utils import run_bass_kernel_spmd
from concourse.masks import make_identity

try:  # best-effort NTFF profiling hook (axon image lacks it by default)
    try:
        from antenv.axon_hooks import (
            get_axon_ntff_profile_hook,
            set_axon_ntff_profile_hook,
        )
    except ImportError:
        # image's antenv lacks axon_hooks -- install a shim module so
        # bass_utils' `from antenv.axon_hooks import ...` resolves
        import sys as _sys
        import types as _types

        import antenv as _antenv

        _mod = _types.ModuleType("antenv.axon_hooks")
        _hook_box = [None]
        _mod.get_axon_ntff_profile_hook = lambda: _hook_box[0]
        _mod.set_axon_ntff_profile_hook = (
            lambda h: _hook_box.__setitem__(0, h)
        )
        _sys.modules["antenv.axon_hooks"] = _mod
        _antenv.axon_hooks = _mod
        get_axon_ntff_profile_hook = _mod.get_axon_ntff_profile_hook
        set_axon_ntff_profile_hook = _mod.set_axon_ntff_profile_hook

    if get_axon_ntff_profile_hook() is None:
        from trn_agent_boot.trn_boot import _ntff_profile_via_ctypes

        set_axon_ntff_profile_hook(
            _ntff_profile_via_ctypes("/opt/axon/libaxon_pjrt.so")
        )
except Exception:
    pass

BF16 = mybir.dt.bfloat16
F32 = mybir.dt.float32
AF = mybir.ActivationFunctionType
ALU = mybir.AluOpType

B, R, C, E = 8, 256, 256, 512
H, D, HID = 8, 64, 128
NCORES = 8
NWAVES = 32  # 512 positions each: (2 r-rows per 32-row group) x 256 c

LAST_EXEC_NS = None
_CACHE = {}


def _build():
    nc = bacc.Bacc(
        "TRN2", target_bir_lowering=False, debug=False, enable_asserts=False
    )
    t = {}
    t["rembT"] = nc.dram_tensor("rembT", [E, R], BF16, kind="ExternalInput")
    t["cembT"] = nc.dram_tensor("cembT", [E, C], BF16, kind="ExternalInput")
    t["cost"] = nc.dram_tensor("cost16", [R, C], BF16, kind="ExternalInput")
    t["keep"] = nc.dram_tensor("keep16", [R, C], BF16, kind="ExternalInput")
    for w in ("wq", "wk", "wv", "wo"):
        t[w] = nc.dram_tensor(w, [E, E], BF16, kind="ExternalInput")
    t["m9"] = nc.dram_tensor("m9", [128, HID], BF16, kind="ExternalInput")
    t["w2"] = nc.dram_tensor("w2", [HID, H], BF16, kind="ExternalInput")
    t["out"] = nc.dram_tensor("out", [R, E], F32, kind="ExternalOutput")
    # DRAM bounce buffers for cross-partition reshapes (DMA cannot stride
    # the SBUF partition dim; DRAM APs are unconstrained)
    t["fb"] = nc.dram_tensor("fbounce", [2, H, 128, C], BF16, kind="Internal")
    t["mb"] = nc.dram_tensor("mbounce", [2, 4, H, 16 * 512], F32, kind="Internal")

    with tile.TileContext(nc) as tc:
        _kernel_body(tc, t)
    nc.compile()
    return nc


def _kernel_body(tc, t):
    nc = tc.nc
    with (
        tc.tile_pool(name="singles", bufs=1) as singles,
        tc.tile_pool(name="hp", bufs=3) as hpool,
        tc.tile_pool(name="pp", bufs=2) as ppool,
        tc.tile_pool(name="yp", bufs=2) as ypool,
        tc.tile_pool(name="mmps", bufs=2, space="PSUM") as mmps,
        tc.tile_pool(name="w1ps", bufs=1, space="PSUM") as w1ps,
        tc.tile_pool(name="w2ps", bufs=2, space="PSUM") as w2ps,
    ):
        # ---- weights/constants to SBUF, split per chunk so compute can
        # start as soon as the first chunks land; wo is loaded last ----
        def wtile(name):
            return singles.tile([128, 4 * E], BF16, tag=name, name=name)

        wq_sb, wk_sb, wv_sb, wo_sb = map(wtile, ("wq", "wk", "wv", "wo"))
        remb_sb = singles.tile([128, 4 * R], BF16, tag="remb")
        cemb_sb = singles.tile([128, 4 * C], BF16, tag="cemb")

        def load_chunks(sb, th, n, eng=None):
            for k in range(4):
                (eng or nc.sync).dma_start(
                    out=sb[:, n * k : n * (k + 1)],
                    in_=th.ap()[128 * k : 128 * (k + 1), :],
                )

        # spread load issue across sync/scalar/gpsimd queues -- the HWDGE
        # dma_start occupies its sequencer ~1us each
        load_chunks(remb_sb, t["rembT"], R)
        load_chunks(wq_sb, t["wq"], E, nc.scalar)
        load_chunks(cemb_sb, t["cembT"], C)
        load_chunks(wk_sb, t["wk"], E, nc.scalar)
        load_chunks(wv_sb, t["wv"], E, nc.gpsimd)
        m9_sb = singles.tile([128, HID], BF16, tag="m9")
        nc.gpsimd.dma_start(out=m9_sb, in_=t["m9"].ap())
        w2_sb = singles.tile([HID, H], BF16, tag="w2")
        nc.gpsimd.dma_start(out=w2_sb, in_=t["w2"].ap())
        keep_sb = singles.tile([128, 2, C], BF16, tag="keep")
        nc.gpsimd.dma_start(
            out=keep_sb, in_=t["keep"].ap().rearrange("(i p) c -> p i c", p=128)
        )
        ident = singles.tile([128, 128], BF16, tag="ident")
        make_identity(nc, ident)

        # ---- QKV projections ----
        qt_sb = singles.tile([128, 4 * R], BF16, tag="qt")  # [hd, r]
        kt_sb = singles.tile([128, 4 * C], BF16, tag="kt")  # [hd, c]
        v_sb = singles.tile([128, 2 * E], BF16, tag="v")    # [c, hd]

        for m in range(4):  # hd chunk
            ps = mmps.tile([128, 512], F32, tag="mm")
            for k in range(4):
                nc.tensor.matmul(
                    ps[:, 0:R],
                    lhsT=wq_sb[:, 512 * k + 128 * m : 512 * k + 128 * (m + 1)],
                    rhs=remb_sb[:, R * k : R * (k + 1)],
                    start=(k == 0), stop=(k == 3),
                )
            nc.scalar.copy(out=qt_sb[:, R * m : R * (m + 1)], in_=ps[:, 0:R])
        for m in range(4):
            ps = mmps.tile([128, 512], F32, tag="mm")
            for k in range(4):
                nc.tensor.matmul(
                    ps[:, 0:C],
                    lhsT=wk_sb[:, 512 * k + 128 * m : 512 * k + 128 * (m + 1)],
                    rhs=cemb_sb[:, C * k : C * (k + 1)],
                    start=(k == 0), stop=(k == 3),
                )
            nc.scalar.copy(out=kt_sb[:, C * m : C * (m + 1)], in_=ps[:, 0:C])
        for cc in range(2):
            ps = mmps.tile([128, 512], F32, tag="mm")
            for k in range(4):
                nc.tensor.matmul(
                    ps,
                    lhsT=cemb_sb[:, C * k + 128 * cc : C * k + 128 * (cc + 1)],
                    rhs=wv_sb[:, 512 * k : 512 * (k + 1)],
                    start=(k == 0), stop=(k == 3),
                )
            nc.vector.tensor_copy(out=v_sb[:, 512 * cc : 512 * (cc + 1)], in_=ps)

        # ---- dot scores -> F -> DRAM bounce -> S4 (m-outer so rchunk 0's
        # collapse overlaps rchunk 1's dots) ----
        # S4[32g+ch, 8192*i + r''*256 + c] = feat_ch[128*i + 32*g + r'', c]
        f_sb = [
            singles.tile([128, 8 * C], BF16, tag=f"f{i}", name=f"f{i}")
            for i in range(2)
        ]
        s4 = [
            singles.tile([128, 8192], BF16, tag=f"s4_{i}", name=f"s4_{i}")
            for i in range(2)
        ]
        for m in range(2):  # r chunk
            for j in range(4):       # qt/kt chunk (2 heads)
                for s in range(2):   # head within chunk
                    h = 2 * j + s
                    ps = mmps.tile([128, 256], F32, tag="mm")
                    nc.tensor.matmul(
                        ps,
                        lhsT=qt_sb[64 * s : 64 * (s + 1),
                                   R * j + 128 * m : R * j + 128 * (m + 1)],
                        rhs=kt_sb[64 * s : 64 * (s + 1), C * j : C * (j + 1)],
                        start=True, stop=True,
                        tile_position=(64 * s, 0),
                    )
                    if h % 2 == 0:
                        nc.scalar.copy(
                            out=f_sb[m][:, C * h : C * (h + 1)], in_=ps
                        )
                    else:
                        nc.vector.tensor_copy(
                            out=f_sb[m][:, C * h : C * (h + 1)], in_=ps
                        )
            # dump F channel-major: fb[m][ch, r_loc, c], then gather to S4
            nc.sync.dma_start(
                out=t["fb"].ap()[m].transpose([1, 0, 2]),
                in_=f_sb[m].rearrange("p (ch c) -> p ch c", ch=8),
            )
            for g in range(4):
                nc.sync.dma_start(
                    out=s4[m][32 * g : 32 * g + 8, :].rearrange(
                        "p (a b) -> p a b", a=32
                    ),
                    in_=t["fb"].ap()[m][:, 32 * g : 32 * (g + 1), :],
                )
                nc.sync.dma_start(
                    out=s4[m][32 * g + 8 : 32 * g + 9, :],
                    in_=t["cost"].ap()[
                        128 * m + 32 * g : 128 * m + 32 * (g + 1), :
                    ],
                )

        # ---- MLP waves (SW-pipelined) + interleaved softmax/AV/proj ----
        l_sb = [
            singles.tile([128, H * C], F32, tag=f"l{i}", name=f"l{i}")
            for i in range(2)
        ]
        # mbig[32g+h', 512*n' + 256*rp + c] = mixed for row (32g+2n'+rp), c
        mbig = singles.tile([128, 16 * 512], F32, tag="mbig")
        pt_sb = [
            singles.tile([128, H * R], BF16, tag=f"pt{cc}", name=f"pt{cc}")
            for cc in range(2)
        ]
        ot_sb = singles.tile([128, 4 * R], BF16, tag="ot")  # [e, r]

        def stage2(n):
            i, np_ = n // 16, n % 16
            wps, h_sb = wave_state[n]
            w2p = w2ps.tile([128, 512], F32, tag="w2", name=f"w2p{n}")
            # one relu per engine; they run concurrently (the 4-way split
            # just serialized per-engine and bounded the wave cadence)
            nc.scalar.activation(
                out=h_sb[:, 0:1024], in_=wps[:, 0:1024], func=AF.Relu
            )
            nc.vector.tensor_scalar_max(
                out=h_sb[:, 1024:2048], in0=wps[:, 1024:2048], scalar1=0.0
            )
            for g in range(4):
                nc.tensor.matmul(
                    w2p[32 * g : 32 * g + 8, :],
                    lhsT=w2_sb,
                    rhs=h_sb[:, 512 * g : 512 * (g + 1)],
                    start=True, stop=True,
                    tile_position=(0, 32 * g),
                )
            mst = mbig[:, 512 * np_ : 512 * (np_ + 1)]
            if n % 2 == 0:
                nc.vector.tensor_copy(out=mst, in_=w2p)
            else:
                nc.scalar.copy(out=mst, in_=w2p)
            if np_ % 8 == 7:
                # half-rchunk scatter via DRAM bounce
                q = (np_ // 8) % 2
                qs = slice(4096 * q, 4096 * (q + 1))
                for g in range(4):
                    nc.sync.dma_start(
                        out=t["mb"].ap()[i][g][:, qs],
                        in_=mbig[32 * g : 32 * g + 8, qs],
                    )
                for g in range(4):
                    src = (
                        t["mb"].ap()[i][g][:, qs]
                        .rearrange("hh (nn rp c) -> hh nn rp c", nn=8, rp=2)
                        .transpose([1, 2, 0, 3])
                    )
                    dst = l_sb[i][
                        32 * g + 16 * q : 32 * g + 16 * (q + 1), :
                    ].rearrange("p (hh c) -> p hh c", hh=H)
                    nc.sync.dma_start(out=dst, in_=src)

        def phase_c(i):
            # softmax (no max-subtraction; multiplicative mask after exp),
            # pipelined per head so the final-rchunk tail stays short:
            # exp_h -> keep-mul+rowsum_h -> recip_h -> scale_h -> 2 PE
            # transposes -> AV for the head pair.
            p_f = ppool.tile([128, H * C], F32, tag="p", name=f"p{i}")
            sums = singles.tile([128, H], F32, tag=f"sums{i}", name=f"sums{i}")
            recips = singles.tile(
                [128, H], F32, tag=f"recips{i}", name=f"recips{i}"
            )
            pb = singles.tile([128, H * C], BF16, tag=f"pb{i}", name=f"pb{i}")
            for hh in range(H):
                hs = slice(C * hh, C * (hh + 1))
                nc.scalar.activation(out=p_f[:, hs], in_=l_sb[i][:, hs],
                                     func=AF.Exp)
                nc.vector.scalar_tensor_tensor(
                    out=pb[:, hs],
                    in0=p_f[:, hs],
                    scalar=1.0,
                    in1=keep_sb[:, i, :],
                    op0=ALU.mult,
                    op1=ALU.mult,
                    accum_out=sums[:, hh : hh + 1],
                )
                nc.vector.tensor_scalar_add(
                    out=sums[:, hh : hh + 1], in0=sums[:, hh : hh + 1],
                    scalar1=1e-30,
                )
                nc.vector.reciprocal(
                    out=recips[:, hh : hh + 1], in_=sums[:, hh : hh + 1]
                )
                nc.vector.tensor_scalar_mul(
                    out=pb[:, hs], in0=pb[:, hs],
                    scalar1=recips[:, hh : hh + 1],
                )
                for cc in range(2):
                    tp = mmps.tile([128, 128], BF16, tag="mm",
                                   name=f"tp{i}_{hh}_{cc}")
                    nc.tensor.transpose(
                        tp,
                        in_=pb[:, C * hh + 128 * cc : C * hh + 128 * (cc + 1)],
                        identity=ident,
                    )
                    dstp = pt_sb[cc][:, R * hh + 128 * i : R * hh + 128 * (i + 1)]
                    if (hh + cc) % 2 == 0:
                        nc.scalar.copy(out=dstp, in_=tp)
                    else:
                        nc.vector.tensor_copy(out=dstp, in_=tp)
                if hh % 2 == 1:
                    # AV for head pair (hh-1, hh), r-half i
                    j = hh // 2
                    ps = mmps.tile([128, 128], F32, tag="mm", name=f"av{i}_{j}")
                    for s in range(2):
                        h = 2 * j + s
                        for cc in range(2):
                            nc.tensor.matmul(
                                ps[64 * s : 64 * (s + 1), :],
                                lhsT=v_sb[:, 512 * cc + 64 * h :
                                          512 * cc + 64 * (h + 1)],
                                rhs=pt_sb[cc][:, R * h + 128 * i :
                                              R * h + 128 * (i + 1)],
                                start=(cc == 0), stop=(cc == 1),
                            )
                    if j % 2 == 0:
                        nc.vector.tensor_copy(
                            out=ot_sb[:, R * j + 128 * i : R * j + 128 * (i + 1)],
                            in_=ps,
                        )
                    else:
                        nc.scalar.copy(
                            out=ot_sb[:, R * j + 128 * i : R * j + 128 * (i + 1)],
                            in_=ps,
                        )

        def tail(i):
            # output projection for r-half i
            ps = mmps.tile([128, 512], F32, tag="mm", name=f"yps{i}")
            for k in range(4):
                nc.tensor.matmul(
                    ps,
                    lhsT=ot_sb[:, R * k + 128 * i : R * k + 128 * (i + 1)],
                    rhs=wo_sb[:, 512 * k : 512 * (k + 1)],
                    start=(k == 0), stop=(k == 3),
                )
            y = ypool.tile([128, 512], F32, tag="y", name=f"y{i}")
            nc.scalar.copy(out=y, in_=ps)
            nc.sync.dma_start(out=t["out"].ap()[128 * i : 128 * (i + 1), :], in_=y)

        load_chunks(wo_sb, t["wo"], E, nc.gpsimd)

        wave_state = {}
        for n in range(NWAVES):
            i, np_ = n // 16, n % 16
            wps = w1ps.tile([128, 2048], F32, tag="w1", name=f"wps{n}")
            for g in range(4):
                nc.tensor.matmul(
                    wps[:, 512 * g : 512 * (g + 1)],
                    lhsT=m9_sb[32 * g : 32 * g + 9, :],
                    rhs=s4[i][32 * g : 32 * g + 9, 512 * np_ : 512 * (np_ + 1)],
                    start=True, stop=True,
                    tile_position=(32 * g, 0),
                )
            wave_state[n] = (
                wps,
                hpool.tile([128, 2048], BF16, tag="h", name=f"h{n}"),
            )
            if n > 0:
                stage2(n - 1)
            if n == 20:
                phase_c(0)
                tail(0)
        stage2(NWAVES - 1)
        phase_c(1)
        tail(1)


def _prep_inputs(row_emb, col_emb, cost_mat, attn_mask, Wq, Wk, Wv, Wo, W1,
                 W2, alpha):
    bf = ml_dtypes.bfloat16
    alpha_v = np.asarray(alpha, np.float32).reshape(-1)  # [H]
    W1 = np.asarray(W1, np.float32)
    # M9 row h (h<8): W1[2h,:]/sqrt(D); row 8: sum_h alpha_h * W1[2h+1,:]
    m9 = np.zeros((128, HID), np.float32)
    for g in range(4):
        for hh in range(H):
            m9[32 * g + hh] = W1[2 * hh] / np.sqrt(D)
        m9[32 * g + 8] = sum(alpha_v[hh] * W1[2 * hh + 1] for hh in range(H))
    shared = {
        "wq": np.asarray(Wq, np.float32).astype(bf),
        "wk": np.asarray(Wk, np.float32).astype(bf),
        "wv": np.asarray(Wv, np.float32).astype(bf),
        "wo": np.asarray(Wo, np.float32).astype(bf),
        "m9": m9.astype(bf),
        "w2": np.asarray(W2, np.float32).astype(bf),
    }
    in_maps = []
    for b in range(B):
        m = dict(shared)
        m["rembT"] = np.ascontiguousarray(
            np.asarray(row_emb[b], np.float32).T
        ).astype(bf)
        m["cembT"] = np.ascontiguousarray(
            np.asarray(col_emb[b], np.float32).T
        ).astype(bf)
        m["cost16"] = np.asarray(cost_mat[b, :, :, 0], np.float32).astype(bf)
        m["keep16"] = (~np.asarray(attn_mask[b])).astype(np.float32).astype(bf)
        in_maps.append(m)
    return in_maps


def kernel(**inputs) -> np.ndarray:
    global LAST_EXEC_NS
    if "nc" not in _CACHE:
        _CACHE["nc"] = _build()
    nc = _CACHE["nc"]
    in_maps = _prep_inputs(**inputs)
    trace = os.environ.get("KERNEL_TRACE", "0") == "1"
    res = run_bass_kernel_spmd(
        nc, in_maps, core_ids=list(range(NCORES)), trace=trace
    )
    LAST_EXEC_NS = res.exec_time_ns
    out = np.stack([np.asarray(res.results[b]["out"]) for b in range(B)])
    return out.astype(np.float32)



# revision 23
# speedup vs baseline: 1.0247x; 1.0247x over previous
"""MixedScoreMultiHeadAttention on 8 TRN2 NeuronCores.

Sharding: data-parallel over batch B=8 (one batch element per core, no
collectives).  Per core (R=C=256, E=512, H=8, D=64, HID=128):

  1. QKV projections (bf16 matmuls; embeddings host-pretransposed to [E, S]).
  2. Per-head dot scores (K=64 matmuls, 2 heads packed via row groups).
  3. Channel-collapse via a DRAM bounce into S4 [32g+ch, pos] so the
     score-MLP runs channel-major with 4x tile_position row-packing (K=9).
  4. MLP waves (software-pipelined): W1 (4 concurrent row-tiled matmuls) ->
     per-group relu evict (ACT+DVE split, the elementwise bottleneck) ->
     W2 (4 concurrent col-tiled M=8 matmuls) -> DRAM-bounce scatter back to
     [r, (h, c)] logit tiles, half-rchunk granularity.
  5. Softmax without max-subtraction (logits are provably O(5)), mask applied
     multiplicatively after exp (fully-masked rows via +eps on the
     denominator), DMA-transpose of the weights, AV producing out^T per
     r-half, final projection per r-half -- all interleaved with the wave
     loop of the other row chunk.

The score-MLP weights are algebraically folded on the host:
  hidden = relu(concat_h[dot_h, alpha_h*cost] @ W1)
         = relu(sum_h dot_h * W1[2h,:] + cost * sum_h alpha_h W1[2h+1,:])
so the device sees a 9-channel input (8 raw-dot channels + 1 cost channel)
and an M9 [9, HID] matrix with the 1/sqrt(D) norm folded into the dot rows.
"""

import os

os.environ.setdefault("MYCRO_LOCAL_CACHE", "1")

import numpy as np
import ml_dtypes

import concourse.bass as bass
import concourse.mybir as mybir
import concourse.tile as tile
from concourse import bacc
from concourse.bass_utils import run_bass_kernel_spmd
from concourse.masks import make_identity

try:  # best-effort NTFF profiling hook (axon image lacks it by default)
    try:
        from antenv.axon_hooks import (
            get_axon_ntff_profile_hook,
            set_axon_ntff_profile_hook,
        )
    except ImportError:
        # image's antenv lacks axon_hooks -- install a shim module so
        # bass_utils' `from antenv.axon_hooks import ...` resolves
        import sys as _sys
        import types as _types

        import antenv as _antenv

        _mod = _types.ModuleType("antenv.axon_hooks")
        _hook_box = [None]
        _mod.get_axon_ntff_profile_hook = lambda: _hook_box[0]
        _mod.set_axon_ntff_profile_hook = (
            lambda h: _hook_box.__setitem__(0, h)
        )
        _sys.modules["antenv.axon_hooks"] = _mod
        _antenv.axon_hooks = _mod
        get_axon_ntff_profile_hook = _mod.get_axon_ntff_profile_hook
        set_axon_ntff_profile_hook = _mod.set_axon_ntff_profile_hook

    if get_axon_ntff_profile_hook() is None:
        from trn_agent_boot.trn_boot import _ntff_profile_via_ctypes

        set_axon_ntff_profile_hook(
            _ntff_profile_via_ctypes("/opt/axon/libaxon_pjrt.so")
        )
except Exception:
    pass

BF16 = mybir.dt.bfloat16
F32 = mybir.dt.float32
AF = mybir.ActivationFunctionType
ALU = mybir.AluOpType

B, R, C, E = 8, 256, 256, 512
H, D, HID = 8, 64, 128
NCORES = 8
NWAVES = 32  # 512 positions each: (2 r-rows per 32-row group) x 256 c
EXP_ON_EVICT = os.environ.get("EXP_ON_EVICT", "1") == "1"

LAST_EXEC_NS = None
_CACHE = {}


def _build():
    nc = bacc.Bacc(
        "TRN2", target_bir_lowering=False, debug=False, enable_asserts=False
    )
    t = {}
    t["rembT"] = nc.dram_tensor("rembT", [E, R], BF16, kind="ExternalInput")
    t["cembT"] = nc.dram_tensor("cembT", [E, C], BF16, kind="ExternalInput")
    t["cost"] = nc.dram_tensor("cost16", [R, C], BF16, kind="ExternalInput")
    t["keep"] = nc.dram_tensor("keep16", [R, C], BF16, kind="ExternalInput")
    for w in ("wq", "wk", "wv", "wo"):
        t[w] = nc.dram_tensor(w, [E, E], BF16, kind="ExternalInput")
    t["m9"] = nc.dram_tensor("m9", [128, HID], BF16, kind="ExternalInput")
    t["w2"] = nc.dram_tensor("w2", [HID, H], BF16, kind="ExternalInput")
    t["out"] = nc.dram_tensor("out", [R, E], F32, kind="ExternalOutput")
    # DRAM bounce buffers for cross-partition reshapes (DMA cannot stride
    # the SBUF partition dim; DRAM APs are unconstrained)
    t["fb"] = nc.dram_tensor("fbounce", [2, H, 128, C], BF16, kind="Internal")
    # holds exp(mixed logits) in bf16, channel-major wave layout; rows
    # 32g+8 .. 32g+31 are junk (full-partition scatter)
    t["mb"] = nc.dram_tensor("mbounce", [2, 128, 16 * 512], BF16, kind="Internal")

    with tile.TileContext(nc) as tc:
        _kernel_body(tc, t)
    nc.compile()
    return nc


def _kernel_body(tc, t):
    nc = tc.nc
    with (
        tc.tile_pool(name="singles", bufs=1) as singles,
        tc.tile_pool(name="hp", bufs=3) as hpool,
        tc.tile_pool(name="msp", bufs=3) as mspool,
        tc.tile_pool(name="yp", bufs=2) as ypool,
        tc.tile_pool(name="mmps", bufs=2, space="PSUM") as mmps,
        tc.tile_pool(name="w1ps", bufs=1, space="PSUM") as w1ps,
        tc.tile_pool(name="w2ps", bufs=2, space="PSUM") as w2ps,
    ):
        # ---- weights/constants to SBUF, split per chunk so compute can
        # start as soon as the first chunks land; wo is loaded last ----
        def wtile(name):
            return singles.tile([128, 4 * E], BF16, tag=name, name=name)

        wq_sb, wk_sb, wv_sb, wo_sb = map(wtile, ("wq", "wk", "wv", "wo"))
        remb_sb = singles.tile([128, 4 * R], BF16, tag="remb")
        cemb_sb = singles.tile([128, 4 * C], BF16, tag="cemb")

        def load_chunks(sb, th, n, eng=None):
            for k in range(4):
                (eng or nc.sync).dma_start(
                    out=sb[:, n * k : n * (k + 1)],
                    in_=th.ap()[128 * k : 128 * (k + 1), :],
                )

        # spread load issue across sync/scalar/gpsimd queues -- the HWDGE
        # dma_start occupies its sequencer ~1us each
        load_chunks(remb_sb, t["rembT"], R)
        load_chunks(wq_sb, t["wq"], E, nc.scalar)
        load_chunks(cemb_sb, t["cembT"], C)
        load_chunks(wk_sb, t["wk"], E, nc.scalar)
        load_chunks(wv_sb, t["wv"], E, nc.gpsimd)
        m9_sb = singles.tile([128, HID], BF16, tag="m9")
        nc.gpsimd.dma_start(out=m9_sb, in_=t["m9"].ap())
        w2_sb = singles.tile([HID, H], BF16, tag="w2")
        nc.gpsimd.dma_start(out=w2_sb, in_=t["w2"].ap())
        keep_sb = singles.tile([128, 2, C], BF16, tag="keep")
        nc.gpsimd.dma_start(
            out=keep_sb, in_=t["keep"].ap().rearrange("(i p) c -> p i c", p=128)
        )
        ident = singles.tile([128, 128], BF16, tag="ident")
        make_identity(nc, ident)
        # preload the exp table set (~2.7us) while the front phase runs so
        # the first wave's fused exp evict doesn't eat the load
        warm = singles.tile([128, 8], F32, tag="warm")
        nc.scalar.activation(out=warm, in_=ident[:, 0:8], func=AF.Exp)

        # ---- QKV projections ----
        qt_sb = singles.tile([128, 4 * R], BF16, tag="qt")  # [hd, r]
        kt_sb = singles.tile([128, 4 * C], BF16, tag="kt")  # [hd, c]
        v_sb = singles.tile([128, 2 * E], BF16, tag="v")    # [c, hd]

        for m in range(4):  # hd chunk
            ps = mmps.tile([128, 512], F32, tag="mm")
            for k in range(4):
                nc.tensor.matmul(
                    ps[:, 0:R],
                    lhsT=wq_sb[:, 512 * k + 128 * m : 512 * k + 128 * (m + 1)],
                    rhs=remb_sb[:, R * k : R * (k + 1)],
                    start=(k == 0), stop=(k == 3),
                )
            nc.scalar.copy(out=qt_sb[:, R * m : R * (m + 1)], in_=ps[:, 0:R])
        for m in range(4):
            ps = mmps.tile([128, 512], F32, tag="mm")
            for k in range(4):
                nc.tensor.matmul(
                    ps[:, 0:C],
                    lhsT=wk_sb[:, 512 * k + 128 * m : 512 * k + 128 * (m + 1)],
                    rhs=cemb_sb[:, C * k : C * (k + 1)],
                    start=(k == 0), stop=(k == 3),
                )
            nc.scalar.copy(out=kt_sb[:, C * m : C * (m + 1)], in_=ps[:, 0:C])
        for cc in range(2):
            ps = mmps.tile([128, 512], F32, tag="mm")
            for k in range(4):
                nc.tensor.matmul(
                    ps,
                    lhsT=cemb_sb[:, C * k + 128 * cc : C * k + 128 * (cc + 1)],
                    rhs=wv_sb[:, 512 * k : 512 * (k + 1)],
                    start=(k == 0), stop=(k == 3),
                )
            nc.vector.tensor_copy(out=v_sb[:, 512 * cc : 512 * (cc + 1)], in_=ps)

        # ---- dot scores -> F -> DRAM bounce -> S4 (m-outer so rchunk 0's
        # collapse overlaps rchunk 1's dots) ----
        # S4[32g+ch, 8192*i + r''*256 + c] = feat_ch[128*i + 32*g + r'', c]
        f_sb = [
            singles.tile([128, 8 * C], BF16, tag=f"f{i}", name=f"f{i}")
            for i in range(2)
        ]
        s4 = [
            singles.tile([128, 8192], BF16, tag=f"s4_{i}", name=f"s4_{i}")
            for i in range(2)
        ]
        for m in range(2):  # r chunk
            for j in range(4):       # qt/kt chunk (2 heads)
                for s in range(2):   # head within chunk
                    h = 2 * j + s
                    ps = mmps.tile([128, 256], F32, tag="mm")
                    nc.tensor.matmul(
                        ps,
                        lhsT=qt_sb[64 * s : 64 * (s + 1),
                                   R * j + 128 * m : R * j + 128 * (m + 1)],
                        rhs=kt_sb[64 * s : 64 * (s + 1), C * j : C * (j + 1)],
                        start=True, stop=True,
                        tile_position=(64 * s, 0),
                    )
                    if h % 2 == 0:
                        nc.scalar.copy(
                            out=f_sb[m][:, C * h : C * (h + 1)], in_=ps
                        )
                    else:
                        nc.vector.tensor_copy(
                            out=f_sb[m][:, C * h : C * (h + 1)], in_=ps
                        )
            # dump F channel-major: fb[m][ch, r_loc, c], then gather to S4
            nc.sync.dma_start(
                out=t["fb"].ap()[m].transpose([1, 0, 2]),
                in_=f_sb[m].rearrange("p (ch c) -> p ch c", ch=8),
            )
            for g in range(4):
                nc.sync.dma_start(
                    out=s4[m][32 * g : 32 * g + 8, :].rearrange(
                        "p (a b) -> p a b", a=32
                    ),
                    in_=t["fb"].ap()[m][:, 32 * g : 32 * (g + 1), :],
                )
                nc.sync.dma_start(
                    out=s4[m][32 * g + 8 : 32 * g + 9, :],
                    in_=t["cost"].ap()[
                        128 * m + 32 * g : 128 * m + 32 * (g + 1), :
                    ],
                )

        # ---- MLP waves (SW-pipelined) + interleaved softmax/AV/proj ----
        # l_sb holds exp(logits) [r, (h, c)] in bf16 (exp applied during the
        # w2p PSUM evict, fused into the mandatory copy)
        l_sb = [
            singles.tile([128, H * C], BF16, tag=f"l{i}", name=f"l{i}")
            for i in range(2)
        ]
        pt_sb = [
            singles.tile([128, H * R], BF16, tag=f"pt{cc}", name=f"pt{cc}")
            for cc in range(2)
        ]
        ot_sb = singles.tile([128, 4 * R], BF16, tag="ot")  # [e, r]

        def expevict(m):
            # evict wave m's mixed scores from PSUM with a fused exp, then
            # scatter to the DRAM bounce; emitted one wave later so the ACT
            # queue never stalls waiting on W2(m)
            i, np_ = m // 16, m % 16
            w2p = wave_w2p[m]
            ms = mspool.tile([128, 512], BF16, tag="ms", name=f"ms{m}")
            if EXP_ON_EVICT:
                nc.scalar.activation(out=ms, in_=w2p, func=AF.Exp)
            else:
                nc.scalar.copy(out=ms, in_=w2p)
            eng = nc.sync if i == 0 else nc.gpsimd
            eng.dma_start(
                out=t["mb"].ap()[i][:, 512 * np_ : 512 * (np_ + 1)],
                in_=ms,
            )
            if np_ % 8 == 7:
                # half-rchunk gather into l_sb [r, (h, c)]
                q = (np_ // 8) % 2
                qs = slice(4096 * q, 4096 * (q + 1))
                for g in range(4):
                    src = (
                        t["mb"].ap()[i][32 * g : 32 * g + H][:, qs]
                        .rearrange("hh (nn rp c) -> hh nn rp c", nn=8, rp=2)
                        .transpose([1, 2, 0, 3])
                    )
                    dst = l_sb[i][
                        32 * g + 16 * q : 32 * g + 16 * (q + 1), :
                    ].rearrange("p (hh c) -> p hh c", hh=H)
                    eng.dma_start(out=dst, in_=src)

        def stage2(n):
            i, np_ = n // 16, n % 16
            wps, h_sb = wave_state[n]
            w2p = w2ps.tile([128, 512], F32, tag="w2", name=f"w2p{n}")
            wave_w2p[n] = w2p
            # relu evict split across ACT/DVE (ACT also carries the exp
            # evict of the previous wave, so DVE gets the bigger share)
            nc.scalar.activation(
                out=h_sb[:, 0:768], in_=wps[:, 0:768], func=AF.Relu
            )
            nc.vector.tensor_scalar_max(
                out=h_sb[:, 768:2048], in0=wps[:, 768:2048], scalar1=0.0
            )
            for g in range(4):
                nc.tensor.matmul(
                    w2p[32 * g : 32 * g + 8, :],
                    lhsT=w2_sb,
                    rhs=h_sb[:, 512 * g : 512 * (g + 1)],
                    start=True, stop=True,
                    tile_position=(0, 32 * g),
                )
            if n > 0:
                expevict(n - 1)

        pc_state = {}

        def phase_c_head(i, hh):
            # softmax for one head (exp already fused into the w2p evict):
            # keep-mul+rowsum_h -> recip_h -> scale_h -> 2 PE transposes ->
            # AV for the head pair on odd hh.
            if i not in pc_state:
                pc_state[i] = (
                    singles.tile([128, H], F32, tag=f"sums{i}",
                                 name=f"sums{i}"),
                    singles.tile([128, H], F32, tag=f"recips{i}",
                                 name=f"recips{i}"),
                    singles.tile([128, H * C], BF16, tag=f"pb{i}",
                                 name=f"pb{i}"),
                )
            sums, recips, pb = pc_state[i]
            hs = slice(C * hh, C * (hh + 1))
            if not EXP_ON_EVICT:
                nc.scalar.activation(out=l_sb[i][:, hs], in_=l_sb[i][:, hs],
                                     func=AF.Exp)
            nc.vector.scalar_tensor_tensor(
                out=pb[:, hs],
                in0=l_sb[i][:, hs],
                scalar=1.0,
                in1=keep_sb[:, i, :],
                op0=ALU.mult,
                op1=ALU.mult,
                accum_out=sums[:, hh : hh + 1],
            )
            nc.vector.tensor_scalar_add(
                out=sums[:, hh : hh + 1], in0=sums[:, hh : hh + 1],
                scalar1=1e-30,
            )
            nc.vector.reciprocal(
                out=recips[:, hh : hh + 1], in_=sums[:, hh : hh + 1]
            )
            nc.vector.tensor_scalar_mul(
                out=pb[:, hs], in0=pb[:, hs],
                scalar1=recips[:, hh : hh + 1],
            )
            for cc in range(2):
                tp = mmps.tile([128, 128], BF16, tag="mm",
                               name=f"tp{i}_{hh}_{cc}")
                nc.tensor.transpose(
                    tp,
                    in_=pb[:, C * hh + 128 * cc : C * hh + 128 * (cc + 1)],
                    identity=ident,
                )
                dstp = pt_sb[cc][:, R * hh + 128 * i : R * hh + 128 * (i + 1)]
                if (hh + cc) % 2 == 0:
                    nc.scalar.copy(out=dstp, in_=tp)
                else:
                    nc.vector.tensor_copy(out=dstp, in_=tp)
            if hh % 2 == 1:
                # AV for head pair (hh-1, hh), r-half i
                j = hh // 2
                ps = mmps.tile([128, 128], F32, tag="mm", name=f"av{i}_{j}")
                for s in range(2):
                    h = 2 * j + s
                    for cc in range(2):
                        nc.tensor.matmul(
                            ps[64 * s : 64 * (s + 1), :],
                            lhsT=v_sb[:, 512 * cc + 64 * h :
                                      512 * cc + 64 * (h + 1)],
                            rhs=pt_sb[cc][:, R * h + 128 * i :
                                          R * h + 128 * (i + 1)],
                            start=(cc == 0), stop=(cc == 1),
                        )
                if j % 2 == 0:
                    nc.vector.tensor_copy(
                        out=ot_sb[:, R * j + 128 * i : R * j + 128 * (i + 1)],
                        in_=ps,
                    )
                else:
                    nc.scalar.copy(
                        out=ot_sb[:, R * j + 128 * i : R * j + 128 * (i + 1)],
                        in_=ps,
                    )

        def tail(i):
            # output projection for r-half i
            ps = mmps.tile([128, 512], F32, tag="mm", name=f"yps{i}")
            for k in range(4):
                nc.tensor.matmul(
                    ps,
                    lhsT=ot_sb[:, R * k + 128 * i : R * k + 128 * (i + 1)],
                    rhs=wo_sb[:, 512 * k : 512 * (k + 1)],
                    start=(k == 0), stop=(k == 3),
                )
            y = ypool.tile([128, 512], F32, tag="y", name=f"y{i}")
            nc.scalar.copy(out=y, in_=ps)
            nc.sync.dma_start(out=t["out"].ap()[128 * i : 128 * (i + 1), :], in_=y)

        load_chunks(wo_sb, t["wo"], E, nc.gpsimd)

        wave_state = {}
        wave_w2p = {}

        def w1wave(n):
            i, np_ = n // 16, n % 16
            wps = w1ps.tile([128, 2048], F32, tag="w1", name=f"wps{n}")
            for g in range(4):
                nc.tensor.matmul(
                    wps[:, 512 * g : 512 * (g + 1)],
                    lhsT=m9_sb[32 * g : 32 * g + 9, :],
                    rhs=s4[i][32 * g : 32 * g + 9, 512 * np_ : 512 * (np_ + 1)],
                    start=True, stop=True,
                    tile_position=(32 * g, 0),
                )
            wave_state[n] = (
                wps,
                hpool.tile([128, 2048], BF16, tag="h", name=f"h{n}"),
            )

        # emission order puts stage2(n-1) (incl. its W2 matmuls) BEFORE
        # w1wave(n) so the strict PE FIFO matches data-readiness order;
        # phase_c(0) is spread one head per wave once l_sb[0] has landed
        w1wave(0)
        for n in range(1, NWAVES):
            stage2(n - 1)
            w1wave(n)
            if 19 <= n <= 26:
                phase_c_head(0, n - 19)
            if n == 27:
                tail(0)
        stage2(NWAVES - 1)
        expevict(NWAVES - 1)
        for hh in range(H):
            phase_c_head(1, hh)
        tail(1)


def _prep_inputs(row_emb, col_emb, cost_mat, attn_mask, Wq, Wk, Wv, Wo, W1,
                 W2, alpha):
    bf = ml_dtypes.bfloat16
    alpha_v = np.asarray(alpha, np.float32).reshape(-1)  # [H]
    W1 = np.asarray(W1, np.float32)
    # M9 row h (h<8): W1[2h,:]/sqrt(D); row 8: sum_h alpha_h * W1[2h+1,:]
    m9 = np.zeros((128, HID), np.float32)
    for g in range(4):
        for hh in range(H):
            m9[32 * g + hh] = W1[2 * hh] / np.sqrt(D)
        m9[32 * g + 8] = sum(alpha_v[hh] * W1[2 * hh + 1] for hh in range(H))
    shared = {
        "wq": np.asarray(Wq, np.float32).astype(bf),
        "wk": np.asarray(Wk, np.float32).astype(bf),
        "wv": np.asarray(Wv, np.float32).astype(bf),
        "wo": np.asarray(Wo, np.float32).astype(bf),
        "m9": m9.astype(bf),
        "w2": np.asarray(W2, np.float32).astype(bf),
    }
    in_maps = []
    for b in range(B):
        m = dict(shared)
        m["rembT"] = np.ascontiguousarray(
            np.asarray(row_emb[b], np.float32).T
        ).astype(bf)
        m["cembT"] = np.ascontiguousarray(
            np.asarray(col_emb[b], np.float32).T
        ).astype(bf)
        m["cost16"] = np.asarray(cost_mat[b, :, :, 0], np.float32).astype(bf)
        m["keep16"] = (~np.asarray(attn_mask[b])).astype(np.float32).astype(bf)
        in_maps.append(m)
    return in_maps


def kernel(**inputs) -> np.ndarray:
    global LAST_EXEC_NS
    if "nc" not in _CACHE:
        _CACHE["nc"] = _build()
    nc = _CACHE["nc"]
    in_maps = _prep_inputs(**inputs)
    trace = os.environ.get("KERNEL_TRACE", "0") == "1"
    res = run_bass_kernel_spmd(
        nc, in_maps, core_ids=list(range(NCORES)), trace=trace
    )
    LAST_EXEC_NS = res.exec_time_ns
    out = np.stack([np.asarray(res.results[b]["out"]) for b in range(B)])
    return out.astype(np.float32)



# revision 27
# speedup vs baseline: 1.0296x; 1.0048x over previous
"""MixedScoreMultiHeadAttention on 8 TRN2 NeuronCores.

Sharding: data-parallel over batch B=8 (one batch element per core, no
collectives).  Per core (R=C=256, E=512, H=8, D=64, HID=128):

  1. QKV projections (bf16 matmuls; embeddings host-pretransposed to [E, S]).
  2. Per-head dot scores (K=64 matmuls, 2 heads packed via row groups).
  3. Channel-collapse via a DRAM bounce into S4 [32g+ch, pos] so the
     score-MLP runs channel-major with 4x tile_position row-packing (K=9).
  4. MLP waves (software-pipelined): W1 (4 concurrent row-tiled matmuls) ->
     per-group relu evict (ACT+DVE split, the elementwise bottleneck) ->
     W2 (4 concurrent col-tiled M=8 matmuls) -> DRAM-bounce scatter back to
     [r, (h, c)] logit tiles, half-rchunk granularity.
  5. Softmax without max-subtraction (logits are provably O(5)), mask applied
     multiplicatively after exp (fully-masked rows via +eps on the
     denominator), DMA-transpose of the weights, AV producing out^T per
     r-half, final projection per r-half -- all interleaved with the wave
     loop of the other row chunk.

The score-MLP weights are algebraically folded on the host:
  hidden = relu(concat_h[dot_h, alpha_h*cost] @ W1)
         = relu(sum_h dot_h * W1[2h,:] + cost * sum_h alpha_h W1[2h+1,:])
so the device sees a 9-channel input (8 raw-dot channels + 1 cost channel)
and an M9 [9, HID] matrix with the 1/sqrt(D) norm folded into the dot rows.
"""

import os

os.environ.setdefault("MYCRO_LOCAL_CACHE", "1")

import numpy as np
import ml_dtypes

import concourse.bass as bass
import concourse.mybir as mybir
import concourse.tile as tile
from concourse import bacc
from concourse.bass_utils import run_bass_kernel_spmd
from concourse.masks import make_identity

try:  # best-effort NTFF profiling hook (axon image lacks it by default)
    try:
        from antenv.axon_hooks import (
            get_axon_ntff_profile_hook,
            set_axon_ntff_profile_hook,
        )
    except ImportError:
        # image's antenv lacks axon_hooks -- install a shim module so
        # bass_utils' `from antenv.axon_hooks import ...` resolves
        import sys as _sys
        import types as _types

        import antenv as _antenv

        _mod = _types.ModuleType("antenv.axon_hooks")
        _hook_box = [None]
        _mod.get_axon_ntff_profile_hook = lambda: _hook_box[0]
        _mod.set_axon_ntff_profile_hook = (
            lambda h: _hook_box.__setitem__(0, h)
        )
        _sys.modules["antenv.axon_hooks"] = _mod
        _antenv.axon_hooks = _mod
        get_axon_ntff_profile_hook = _mod.get_axon_ntff_profile_hook
        set_axon_ntff_profile_hook = _mod.set_axon_ntff_profile_hook

    if get_axon_ntff_profile_hook() is None:
        from trn_agent_boot.trn_boot import _ntff_profile_via_ctypes

        set_axon_ntff_profile_hook(
            _ntff_profile_via_ctypes("/opt/axon/libaxon_pjrt.so")
        )
except Exception:
    pass

BF16 = mybir.dt.bfloat16
F32 = mybir.dt.float32
AF = mybir.ActivationFunctionType
ALU = mybir.AluOpType

B, R, C, E = 8, 256, 256, 512
H, D, HID = 8, 64, 128
NCORES = 8
NWAVES = 32  # 512 positions each: (2 r-rows per 32-row group) x 256 c
EXP_ON_EVICT = os.environ.get("EXP_ON_EVICT", "1") == "1"

LAST_EXEC_NS = None
_CACHE = {}


def _build():
    nc = bacc.Bacc(
        "TRN2", target_bir_lowering=False, debug=False, enable_asserts=False
    )
    t = {}
    t["rembT"] = nc.dram_tensor("rembT", [E, R], BF16, kind="ExternalInput")
    t["cembT"] = nc.dram_tensor("cembT", [E, C], BF16, kind="ExternalInput")
    t["cost"] = nc.dram_tensor("cost16", [R, C], BF16, kind="ExternalInput")
    t["keep"] = nc.dram_tensor("keep16", [R, C], BF16, kind="ExternalInput")
    for w in ("wq", "wk", "wv", "wo"):
        t[w] = nc.dram_tensor(w, [E, E], BF16, kind="ExternalInput")
    t["m9"] = nc.dram_tensor("m9", [128, HID], BF16, kind="ExternalInput")
    t["w2"] = nc.dram_tensor("w2", [HID, H], BF16, kind="ExternalInput")
    t["out"] = nc.dram_tensor("out", [R, E], F32, kind="ExternalOutput")
    # DRAM bounce buffers for cross-partition reshapes (DMA cannot stride
    # the SBUF partition dim; DRAM APs are unconstrained)
    t["fb"] = nc.dram_tensor("fbounce", [2, H, 128, C], BF16, kind="Internal")
    # holds exp(mixed logits) in bf16, channel-major wave layout; rows
    # 32g+8 .. 32g+31 are junk (full-partition scatter)
    t["mb"] = nc.dram_tensor("mbounce", [2, 128, 16 * 512], BF16, kind="Internal")

    with tile.TileContext(nc) as tc:
        _kernel_body(tc, t)
    nc.compile()
    return nc


def _kernel_body(tc, t):
    nc = tc.nc
    with (
        tc.tile_pool(name="singles", bufs=1) as singles,
        tc.tile_pool(name="hp", bufs=3) as hpool,
        tc.tile_pool(name="msp", bufs=3) as mspool,
        tc.tile_pool(name="yp", bufs=2) as ypool,
        tc.tile_pool(name="mmps", bufs=2, space="PSUM") as mmps,
        tc.tile_pool(name="w1ps", bufs=1, space="PSUM") as w1ps,
        tc.tile_pool(name="w2ps", bufs=2, space="PSUM") as w2ps,
    ):
        # ---- weights/constants to SBUF, split per chunk so compute can
        # start as soon as the first chunks land; wo is loaded last ----
        def wtile(name):
            return singles.tile([128, 4 * E], BF16, tag=name, name=name)

        wq_sb, wk_sb, wv_sb, wo_sb = map(wtile, ("wq", "wk", "wv", "wo"))
        remb_sb = singles.tile([128, 4 * R], BF16, tag="remb")
        cemb_sb = singles.tile([128, 4 * C], BF16, tag="cemb")

        def load_chunks(sb, th, n, eng=None):
            for k in range(4):
                (eng or nc.sync).dma_start(
                    out=sb[:, n * k : n * (k + 1)],
                    in_=th.ap()[128 * k : 128 * (k + 1), :],
                )

        # spread load issue across sync/scalar/gpsimd queues -- the HWDGE
        # dma_start occupies its sequencer ~1us each
        load_chunks(remb_sb, t["rembT"], R)
        load_chunks(wq_sb, t["wq"], E, nc.scalar)
        load_chunks(cemb_sb, t["cembT"], C)
        load_chunks(wk_sb, t["wk"], E, nc.scalar)
        load_chunks(wv_sb, t["wv"], E, nc.gpsimd)
        m9_sb = singles.tile([128, HID], BF16, tag="m9")
        nc.gpsimd.dma_start(out=m9_sb, in_=t["m9"].ap())
        w2_sb = singles.tile([HID, H], BF16, tag="w2")
        nc.gpsimd.dma_start(out=w2_sb, in_=t["w2"].ap())
        keep_sb = singles.tile([128, 2, C], BF16, tag="keep")
        nc.gpsimd.dma_start(
            out=keep_sb, in_=t["keep"].ap().rearrange("(i p) c -> p i c", p=128)
        )
        ident = singles.tile([128, 128], BF16, tag="ident")
        make_identity(nc, ident)
        # preload the exp table set (~2.7us) while the front phase runs so
        # the first wave's fused exp evict doesn't eat the load
        warm = singles.tile([128, 8], F32, tag="warm")
        nc.scalar.activation(out=warm, in_=ident[:, 0:8], func=AF.Exp)

        # ---- QKV projections ----
        qt_sb = singles.tile([128, 4 * R], BF16, tag="qt")  # [hd, r]
        kt_sb = singles.tile([128, 4 * C], BF16, tag="kt")  # [hd, c]
        v_sb = singles.tile([128, 2 * E], BF16, tag="v")    # [c, hd]

        for m in range(4):  # hd chunk
            ps = mmps.tile([128, 512], F32, tag="mm")
            for k in range(4):
                nc.tensor.matmul(
                    ps[:, 0:R],
                    lhsT=wq_sb[:, 512 * k + 128 * m : 512 * k + 128 * (m + 1)],
                    rhs=remb_sb[:, R * k : R * (k + 1)],
                    start=(k == 0), stop=(k == 3),
                )
            nc.scalar.copy(out=qt_sb[:, R * m : R * (m + 1)], in_=ps[:, 0:R])
        for m in range(4):
            ps = mmps.tile([128, 512], F32, tag="mm")
            for k in range(4):
                nc.tensor.matmul(
                    ps[:, 0:C],
                    lhsT=wk_sb[:, 512 * k + 128 * m : 512 * k + 128 * (m + 1)],
                    rhs=cemb_sb[:, C * k : C * (k + 1)],
                    start=(k == 0), stop=(k == 3),
                )
            nc.scalar.copy(out=kt_sb[:, C * m : C * (m + 1)], in_=ps[:, 0:C])
        for cc in range(2):
            ps = mmps.tile([128, 512], F32, tag="mm")
            for k in range(4):
                nc.tensor.matmul(
                    ps,
                    lhsT=cemb_sb[:, C * k + 128 * cc : C * k + 128 * (cc + 1)],
                    rhs=wv_sb[:, 512 * k : 512 * (k + 1)],
                    start=(k == 0), stop=(k == 3),
                )
            nc.vector.tensor_copy(out=v_sb[:, 512 * cc : 512 * (cc + 1)], in_=ps)

        # ---- dot scores -> F -> DRAM bounce -> S4 (m-outer so rchunk 0's
        # collapse overlaps rchunk 1's dots) ----
        # S4[32g+ch, 8192*i + r''*256 + c] = feat_ch[128*i + 32*g + r'', c]
        f_sb = [
            singles.tile([128, 8 * C], BF16, tag=f"f{i}", name=f"f{i}")
            for i in range(2)
        ]
        s4 = [
            singles.tile([128, 8192], BF16, tag=f"s4_{i}", name=f"s4_{i}")
            for i in range(2)
        ]
        for m in range(2):  # r chunk
            for j in range(4):       # qt/kt chunk (2 heads)
                for s in range(2):   # head within chunk
                    h = 2 * j + s
                    ps = mmps.tile([128, 256], F32, tag="mm")
                    nc.tensor.matmul(
                        ps,
                        lhsT=qt_sb[64 * s : 64 * (s + 1),
                                   R * j + 128 * m : R * j + 128 * (m + 1)],
                        rhs=kt_sb[64 * s : 64 * (s + 1), C * j : C * (j + 1)],
                        start=True, stop=True,
                        tile_position=(64 * s, 0),
                    )
                    if h % 2 == 0:
                        nc.scalar.copy(
                            out=f_sb[m][:, C * h : C * (h + 1)], in_=ps
                        )
                    else:
                        nc.vector.tensor_copy(
                            out=f_sb[m][:, C * h : C * (h + 1)], in_=ps
                        )
            # dump F channel-major: fb[m][ch, r_loc, c], then gather to S4
            nc.sync.dma_start(
                out=t["fb"].ap()[m].transpose([1, 0, 2]),
                in_=f_sb[m].rearrange("p (ch c) -> p ch c", ch=8),
            )
            for g in range(4):
                nc.sync.dma_start(
                    out=s4[m][32 * g : 32 * g + 8, :].rearrange(
                        "p (a b) -> p a b", a=32
                    ),
                    in_=t["fb"].ap()[m][:, 32 * g : 32 * (g + 1), :],
                )
                nc.sync.dma_start(
                    out=s4[m][32 * g + 8 : 32 * g + 9, :],
                    in_=t["cost"].ap()[
                        128 * m + 32 * g : 128 * m + 32 * (g + 1), :
                    ],
                )

        # ---- MLP waves (SW-pipelined) + interleaved softmax/AV/proj ----
        # l_sb holds exp(logits) [r, (h, c)] in bf16 (exp applied during the
        # w2p PSUM evict, fused into the mandatory copy)
        l_sb = [
            singles.tile([128, H * C], BF16, tag=f"l{i}", name=f"l{i}")
            for i in range(2)
        ]
        pt_sb = [
            singles.tile([128, H * R], BF16, tag=f"pt{cc}", name=f"pt{cc}")
            for cc in range(2)
        ]
        ot_sb = singles.tile([128, 4 * R], BF16, tag="ot")  # [e, r]

        def expevict(m):
            # evict wave m's mixed scores from PSUM with a fused exp, then
            # scatter to the DRAM bounce; emitted one wave later so the ACT
            # queue never stalls waiting on W2(m)
            i, np_ = m // 16, m % 16
            w2p = wave_w2p[m]
            ms = mspool.tile([128, 512], BF16, tag="ms", name=f"ms{m}")
            if EXP_ON_EVICT:
                nc.scalar.activation(out=ms, in_=w2p, func=AF.Exp)
            else:
                nc.scalar.copy(out=ms, in_=w2p)
            eng = nc.sync if i == 0 else nc.gpsimd
            eng.dma_start(
                out=t["mb"].ap()[i][:, 512 * np_ : 512 * (np_ + 1)],
                in_=ms,
            )
            if np_ % 8 == 7:
                # half-rchunk gather into l_sb [r, (h, c)]
                q = (np_ // 8) % 2
                qs = slice(4096 * q, 4096 * (q + 1))
                for g in range(4):
                    src = (
                        t["mb"].ap()[i][32 * g : 32 * g + H][:, qs]
                        .rearrange("hh (nn rp c) -> hh nn rp c", nn=8, rp=2)
                        .transpose([1, 2, 0, 3])
                    )
                    dst = l_sb[i][
                        32 * g + 16 * q : 32 * g + 16 * (q + 1), :
                    ].rearrange("p (hh c) -> p hh c", hh=H)
                    eng.dma_start(out=dst, in_=src)

        def stage2(n):
            i, np_ = n // 16, n % 16
            wps, ha, hd = wave_state[n]
            w2p = w2ps.tile([128, 512], F32, tag="w2", name=f"w2p{n}")
            wave_w2p[n] = w2p
            nc.scalar.activation(
                out=ha, in_=wps[:, 0:1024], func=AF.Relu
            )
            nc.vector.tensor_scalar_max(
                out=hd, in0=wps[:, 1024:2048], scalar1=0.0
            )
            for g in range(4):
                nc.tensor.matmul(
                    w2p[32 * g : 32 * g + 8, :],
                    lhsT=w2_sb,
                    rhs=(ha if g < 2 else hd)[:, 512 * (g % 2) : 512 * (g % 2 + 1)],
                    start=True, stop=True,
                    tile_position=(0, 32 * g),
                )
            if n > 0:
                expevict(n - 1)

        pc_state = {}

        def phase_c_head(i, hh):
            # softmax for one head (exp already fused into the w2p evict):
            # keep-mul+rowsum_h -> recip_h -> scale_h -> 2 PE transposes ->
            # AV for the head pair on odd hh.
            if i not in pc_state:
                pc_state[i] = (
                    singles.tile([128, H], F32, tag=f"sums{i}",
                                 name=f"sums{i}"),
                    singles.tile([128, H], F32, tag=f"recips{i}",
                                 name=f"recips{i}"),
                    singles.tile([128, H * C], BF16, tag=f"pb{i}",
                                 name=f"pb{i}"),
                )
            sums, recips, pb = pc_state[i]
            hs = slice(C * hh, C * (hh + 1))
            if not EXP_ON_EVICT:
                nc.scalar.activation(out=l_sb[i][:, hs], in_=l_sb[i][:, hs],
                                     func=AF.Exp)
            nc.vector.scalar_tensor_tensor(
                out=pb[:, hs],
                in0=l_sb[i][:, hs],
                scalar=1.0,
                in1=keep_sb[:, i, :],
                op0=ALU.mult,
                op1=ALU.mult,
                accum_out=sums[:, hh : hh + 1],
            )
            nc.vector.tensor_scalar_add(
                out=sums[:, hh : hh + 1], in0=sums[:, hh : hh + 1],
                scalar1=1e-30,
            )
            nc.vector.reciprocal(
                out=recips[:, hh : hh + 1], in_=sums[:, hh : hh + 1]
            )
            nc.vector.tensor_scalar_mul(
                out=pb[:, hs], in0=pb[:, hs],
                scalar1=recips[:, hh : hh + 1],
            )
            for cc in range(2):
                tp = mmps.tile([128, 128], BF16, tag="mm",
                               name=f"tp{i}_{hh}_{cc}")
                nc.tensor.transpose(
                    tp,
                    in_=pb[:, C * hh + 128 * cc : C * hh + 128 * (cc + 1)],
                    identity=ident,
                )
                dstp = pt_sb[cc][:, R * hh + 128 * i : R * hh + 128 * (i + 1)]
                if (hh + cc) % 2 == 0:
                    nc.scalar.copy(out=dstp, in_=tp)
                else:
                    nc.vector.tensor_copy(out=dstp, in_=tp)
            if hh % 2 == 1:
                # AV for head pair (hh-1, hh), r-half i
                j = hh // 2
                ps = mmps.tile([128, 128], F32, tag="mm", name=f"av{i}_{j}")
                for s in range(2):
                    h = 2 * j + s
                    for cc in range(2):
                        nc.tensor.matmul(
                            ps[64 * s : 64 * (s + 1), :],
                            lhsT=v_sb[:, 512 * cc + 64 * h :
                                      512 * cc + 64 * (h + 1)],
                            rhs=pt_sb[cc][:, R * h + 128 * i :
                                          R * h + 128 * (i + 1)],
                            start=(cc == 0), stop=(cc == 1),
                        )
                if j % 2 == 0:
                    nc.vector.tensor_copy(
                        out=ot_sb[:, R * j + 128 * i : R * j + 128 * (i + 1)],
                        in_=ps,
                    )
                else:
                    nc.scalar.copy(
                        out=ot_sb[:, R * j + 128 * i : R * j + 128 * (i + 1)],
                        in_=ps,
                    )

        def tail(i):
            # output projection for r-half i
            ps = mmps.tile([128, 512], F32, tag="mm", name=f"yps{i}")
            for k in range(4):
                nc.tensor.matmul(
                    ps,
                    lhsT=ot_sb[:, R * k + 128 * i : R * k + 128 * (i + 1)],
                    rhs=wo_sb[:, 512 * k : 512 * (k + 1)],
                    start=(k == 0), stop=(k == 3),
                )
            y = ypool.tile([128, 512], F32, tag="y", name=f"y{i}")
            nc.scalar.copy(out=y, in_=ps)
            nc.sync.dma_start(out=t["out"].ap()[128 * i : 128 * (i + 1), :], in_=y)

        load_chunks(wo_sb, t["wo"], E, nc.gpsimd)

        wave_state = {}
        wave_w2p = {}

        def w1wave(n):
            i, np_ = n // 16, n % 16
            wps = w1ps.tile([128, 2048], F32, tag="w1", name=f"wps{n}")
            for g in range(4):
                nc.tensor.matmul(
                    wps[:, 512 * g : 512 * (g + 1)],
                    lhsT=m9_sb[32 * g : 32 * g + 9, :],
                    rhs=s4[i][32 * g : 32 * g + 9, 512 * np_ : 512 * (np_ + 1)],
                    start=True, stop=True,
                    tile_position=(32 * g, 0),
                )
            # two separate h tiles so the ACT and DVE halves of the relu
            # evict don't serialize on a same-tile WAW dependency
            wave_state[n] = (
                wps,
                hpool.tile([128, 1024], BF16, tag="ha", name=f"ha{n}"),
                hpool.tile([128, 1024], BF16, tag="hd", name=f"hd{n}"),
            )

        # emission order puts stage2(n-1) (incl. its W2 matmuls) BEFORE
        # w1wave(n) so the strict PE FIFO matches data-readiness order;
        # phase_c(0) is spread one head per wave once l_sb[0] has landed
        w1wave(0)
        for n in range(1, NWAVES):
            stage2(n - 1)
            w1wave(n)
            if 19 <= n <= 26:
                phase_c_head(0, n - 19)
            if n == 27:
                tail(0)
        stage2(NWAVES - 1)
        expevict(NWAVES - 1)
        for hh in range(H):
            phase_c_head(1, hh)
        tail(1)


def _prep_inputs(row_emb, col_emb, cost_mat, attn_mask, Wq, Wk, Wv, Wo, W1,
                 W2, alpha):
    bf = ml_dtypes.bfloat16
    alpha_v = np.asarray(alpha, np.float32).reshape(-1)  # [H]
    W1 = np.asarray(W1, np.float32)
    # M9 row h (h<8): W1[2h,:]/sqrt(D); row 8: sum_h alpha_h * W1[2h+1,:]
    m9 = np.zeros((128, HID), np.float32)
    for g in range(4):
        for hh in range(H):
            m9[32 * g + hh] = W1[2 * hh] / np.sqrt(D)
        m9[32 * g + 8] = sum(alpha_v[hh] * W1[2 * hh + 1] for hh in range(H))
    shared = {
        "wq": np.asarray(Wq, np.float32).astype(bf),
        "wk": np.asarray(Wk, np.float32).astype(bf),
        "wv": np.asarray(Wv, np.float32).astype(bf),
        "wo": np.asarray(Wo, np.float32).astype(bf),
        "m9": m9.astype(bf),
        "w2": np.asarray(W2, np.float32).astype(bf),
    }
    in_maps = []
    for b in range(B):
        m = dict(shared)
        m["rembT"] = np.ascontiguousarray(
            np.asarray(row_emb[b], np.float32).T
        ).astype(bf)
        m["cembT"] = np.ascontiguousarray(
            np.asarray(col_emb[b], np.float32).T
        ).astype(bf)
        m["cost16"] = np.asarray(cost_mat[b, :, :, 0], np.float32).astype(bf)
        m["keep16"] = (~np.asarray(attn_mask[b])).astype(np.float32).astype(bf)
        in_maps.append(m)
    return in_maps


def kernel(**inputs) -> np.ndarray:
    global LAST_EXEC_NS
    if "nc" not in _CACHE:
        _CACHE["nc"] = _build()
    nc = _CACHE["nc"]
    in_maps = _prep_inputs(**inputs)
    trace = os.environ.get("KERNEL_TRACE", "0") == "1"
    res = run_bass_kernel_spmd(
        nc, in_maps, core_ids=list(range(NCORES)), trace=trace
    )
    LAST_EXEC_NS = res.exec_time_ns
    out = np.stack([np.asarray(res.results[b]["out"]) for b in range(B)])
    return out.astype(np.float32)



# revision 29
# speedup vs baseline: 1.1968x; 1.1624x over previous
"""MixedScoreMultiHeadAttention on 8 TRN2 NeuronCores.

Sharding: data-parallel over batch B=8 (one batch element per core, no
collectives).  Per core (R=C=256, E=512, H=8, D=64, HID=128):

  1. QKV projections (bf16 matmuls; embeddings host-pretransposed to [E, S]).
  2. Per-head dot scores (K=64 matmuls, 2 heads packed via row groups).
  3. Channel-collapse via a DRAM bounce into S4 [32g+ch, pos] so the
     score-MLP runs channel-major with 4x tile_position row-packing (K=9).
  4. MLP waves (software-pipelined): W1 (4 concurrent row-tiled matmuls) ->
     per-group relu evict (ACT+DVE split, the elementwise bottleneck) ->
     W2 (4 concurrent col-tiled M=8 matmuls) -> DRAM-bounce scatter back to
     [r, (h, c)] logit tiles, half-rchunk granularity.
  5. Softmax without max-subtraction (logits are provably O(5)), mask applied
     multiplicatively after exp (fully-masked rows via +eps on the
     denominator), DMA-transpose of the weights, AV producing out^T per
     r-half, final projection per r-half -- all interleaved with the wave
     loop of the other row chunk.

The score-MLP weights are algebraically folded on the host:
  hidden = relu(concat_h[dot_h, alpha_h*cost] @ W1)
         = relu(sum_h dot_h * W1[2h,:] + cost * sum_h alpha_h W1[2h+1,:])
so the device sees a 9-channel input (8 raw-dot channels + 1 cost channel)
and an M9 [9, HID] matrix with the 1/sqrt(D) norm folded into the dot rows.
"""

import os

os.environ.setdefault("MYCRO_LOCAL_CACHE", "1")

import numpy as np
import ml_dtypes

import concourse.bass as bass
import concourse.mybir as mybir
import concourse.tile as tile
from concourse import bacc
from concourse.bass_utils import run_bass_kernel_spmd
from concourse.masks import make_identity

try:  # best-effort NTFF profiling hook (axon image lacks it by default)
    try:
        from antenv.axon_hooks import (
            get_axon_ntff_profile_hook,
            set_axon_ntff_profile_hook,
        )
    except ImportError:
        # image's antenv lacks axon_hooks -- install a shim module so
        # bass_utils' `from antenv.axon_hooks import ...` resolves
        import sys as _sys
        import types as _types

        import antenv as _antenv

        _mod = _types.ModuleType("antenv.axon_hooks")
        _hook_box = [None]
        _mod.get_axon_ntff_profile_hook = lambda: _hook_box[0]
        _mod.set_axon_ntff_profile_hook = (
            lambda h: _hook_box.__setitem__(0, h)
        )
        _sys.modules["antenv.axon_hooks"] = _mod
        _antenv.axon_hooks = _mod
        get_axon_ntff_profile_hook = _mod.get_axon_ntff_profile_hook
        set_axon_ntff_profile_hook = _mod.set_axon_ntff_profile_hook

    if get_axon_ntff_profile_hook() is None:
        from trn_agent_boot.trn_boot import _ntff_profile_via_ctypes

        set_axon_ntff_profile_hook(
            _ntff_profile_via_ctypes("/opt/axon/libaxon_pjrt.so")
        )
except Exception:
    pass

BF16 = mybir.dt.bfloat16
F32 = mybir.dt.float32
AF = mybir.ActivationFunctionType
ALU = mybir.AluOpType

B, R, C, E = 8, 256, 256, 512
H, D, HID = 8, 64, 128
NCORES = 8
NWAVES = 32  # 512 positions each: (2 r-rows per 32-row group) x 256 c
EXP_ON_EVICT = os.environ.get("EXP_ON_EVICT", "1") == "1"

LAST_EXEC_NS = None
_CACHE = {}


def _build():
    nc = bacc.Bacc(
        "TRN2", target_bir_lowering=False, debug=False, enable_asserts=False
    )
    t = {}
    t["rembT"] = nc.dram_tensor("rembT", [E, R], BF16, kind="ExternalInput")
    t["cembT"] = nc.dram_tensor("cembT", [E, C], BF16, kind="ExternalInput")
    t["cost"] = nc.dram_tensor("cost16", [R, C], BF16, kind="ExternalInput")
    t["keep"] = nc.dram_tensor("keep16", [R, C], BF16, kind="ExternalInput")
    for w in ("wq", "wk", "wv", "wo"):
        t[w] = nc.dram_tensor(w, [E, E], BF16, kind="ExternalInput")
    t["m9"] = nc.dram_tensor("m9", [128, HID], BF16, kind="ExternalInput")
    t["w2"] = nc.dram_tensor("w2", [HID, H], BF16, kind="ExternalInput")
    t["out"] = nc.dram_tensor("out", [R, E], F32, kind="ExternalOutput")
    # DRAM bounce buffers for cross-partition reshapes (DMA cannot stride
    # the SBUF partition dim; DRAM APs are unconstrained)
    t["fb"] = nc.dram_tensor("fbounce", [2, H, 128, C], BF16, kind="Internal")
    # holds exp(mixed logits) in bf16, channel-major wave layout; rows
    # 32g+8 .. 32g+31 are junk (full-partition scatter)
    t["mb"] = nc.dram_tensor("mbounce", [2, 128, 16 * 512], BF16, kind="Internal")

    with tile.TileContext(nc) as tc:
        _kernel_body(tc, t)
    nc.compile()
    return nc


def _kernel_body(tc, t):
    nc = tc.nc
    with (
        tc.tile_pool(name="singles", bufs=1) as singles,
        tc.tile_pool(name="hp", bufs=3) as hpool,
        tc.tile_pool(name="msp", bufs=3) as mspool,
        tc.tile_pool(name="yp", bufs=2) as ypool,
        tc.tile_pool(name="mmps", bufs=2, space="PSUM") as mmps,
        tc.tile_pool(name="w1ps", bufs=1, space="PSUM") as w1ps,
        tc.tile_pool(name="w2ps", bufs=2, space="PSUM") as w2ps,
    ):
        # ---- weights/constants to SBUF, split per chunk so compute can
        # start as soon as the first chunks land; wo is loaded last ----
        def wtile(name):
            return singles.tile([128, 4 * E], BF16, tag=name, name=name)

        wq_sb, wk_sb, wv_sb, wo_sb = map(wtile, ("wq", "wk", "wv", "wo"))
        remb_sb = singles.tile([128, 4 * R], BF16, tag="remb")
        cemb_sb = singles.tile([128, 4 * C], BF16, tag="cemb")

        def load_chunks(sb, th, n, eng=None):
            for k in range(4):
                (eng or nc.sync).dma_start(
                    out=sb[:, n * k : n * (k + 1)],
                    in_=th.ap()[128 * k : 128 * (k + 1), :],
                )

        # spread load issue across sync/scalar/gpsimd queues -- the HWDGE
        # dma_start occupies its sequencer ~1us each
        load_chunks(remb_sb, t["rembT"], R)
        load_chunks(wq_sb, t["wq"], E, nc.scalar)
        load_chunks(cemb_sb, t["cembT"], C)
        load_chunks(wk_sb, t["wk"], E, nc.scalar)
        load_chunks(wv_sb, t["wv"], E, nc.gpsimd)
        m9_sb = singles.tile([128, HID], BF16, tag="m9")
        nc.gpsimd.dma_start(out=m9_sb, in_=t["m9"].ap())
        w2_sb = singles.tile([HID, H], BF16, tag="w2")
        nc.gpsimd.dma_start(out=w2_sb, in_=t["w2"].ap())
        keep_sb = singles.tile([128, 2, C], BF16, tag="keep")
        nc.gpsimd.dma_start(
            out=keep_sb, in_=t["keep"].ap().rearrange("(i p) c -> p i c", p=128)
        )
        ident = singles.tile([128, 128], BF16, tag="ident")
        make_identity(nc, ident)
        # preload the exp table set (~2.7us) while the front phase runs so
        # the first wave's fused exp evict doesn't eat the load
        warm = singles.tile([128, 8], F32, tag="warm")
        nc.scalar.activation(out=warm, in_=ident[:, 0:8], func=AF.Exp)

        # ---- QKV projections ----
        qt_sb = singles.tile([128, 4 * R], BF16, tag="qt")  # [hd, r]
        kt_sb = singles.tile([128, 4 * C], BF16, tag="kt")  # [hd, c]
        v_sb = singles.tile([128, 2 * E], BF16, tag="v")    # [c, hd]

        for m in range(4):  # hd chunk
            ps = mmps.tile([128, 512], F32, tag="mm")
            for k in range(4):
                nc.tensor.matmul(
                    ps[:, 0:R],
                    lhsT=wq_sb[:, 512 * k + 128 * m : 512 * k + 128 * (m + 1)],
                    rhs=remb_sb[:, R * k : R * (k + 1)],
                    start=(k == 0), stop=(k == 3),
                )
            nc.scalar.copy(out=qt_sb[:, R * m : R * (m + 1)], in_=ps[:, 0:R])
        for m in range(4):
            ps = mmps.tile([128, 512], F32, tag="mm")
            for k in range(4):
                nc.tensor.matmul(
                    ps[:, 0:C],
                    lhsT=wk_sb[:, 512 * k + 128 * m : 512 * k + 128 * (m + 1)],
                    rhs=cemb_sb[:, C * k : C * (k + 1)],
                    start=(k == 0), stop=(k == 3),
                )
            nc.scalar.copy(out=kt_sb[:, C * m : C * (m + 1)], in_=ps[:, 0:C])
        for cc in range(2):
            ps = mmps.tile([128, 512], F32, tag="mm")
            for k in range(4):
                nc.tensor.matmul(
                    ps,
                    lhsT=cemb_sb[:, C * k + 128 * cc : C * k + 128 * (cc + 1)],
                    rhs=wv_sb[:, 512 * k : 512 * (k + 1)],
                    start=(k == 0), stop=(k == 3),
                )
            nc.vector.tensor_copy(out=v_sb[:, 512 * cc : 512 * (cc + 1)], in_=ps)

        # ---- dot scores -> F -> DRAM bounce -> S4 (m-outer so rchunk 0's
        # collapse overlaps rchunk 1's dots) ----
        # S4[32g+ch, 8192*i + r''*256 + c] = feat_ch[128*i + 32*g + r'', c]
        f_sb = [
            singles.tile([128, 8 * C], BF16, tag=f"f{i}", name=f"f{i}")
            for i in range(2)
        ]
        s4 = [
            singles.tile([128, 8192], BF16, tag=f"s4_{i}", name=f"s4_{i}")
            for i in range(2)
        ]
        for m in range(2):  # r chunk
            for j in range(4):       # qt/kt chunk (2 heads)
                for s in range(2):   # head within chunk
                    h = 2 * j + s
                    ps = mmps.tile([128, 256], F32, tag="mm")
                    nc.tensor.matmul(
                        ps,
                        lhsT=qt_sb[64 * s : 64 * (s + 1),
                                   R * j + 128 * m : R * j + 128 * (m + 1)],
                        rhs=kt_sb[64 * s : 64 * (s + 1), C * j : C * (j + 1)],
                        start=True, stop=True,
                        tile_position=(64 * s, 0),
                    )
                    if h % 2 == 0:
                        nc.scalar.copy(
                            out=f_sb[m][:, C * h : C * (h + 1)], in_=ps
                        )
                    else:
                        nc.vector.tensor_copy(
                            out=f_sb[m][:, C * h : C * (h + 1)], in_=ps
                        )
            # dump F channel-major: fb[m][ch, r_loc, c], then gather to S4
            nc.sync.dma_start(
                out=t["fb"].ap()[m].transpose([1, 0, 2]),
                in_=f_sb[m].rearrange("p (ch c) -> p ch c", ch=8),
            )
            for g in range(4):
                nc.sync.dma_start(
                    out=s4[m][32 * g : 32 * g + 8, :].rearrange(
                        "p (a b) -> p a b", a=32
                    ),
                    in_=t["fb"].ap()[m][:, 32 * g : 32 * (g + 1), :],
                )
                nc.sync.dma_start(
                    out=s4[m][32 * g + 8 : 32 * g + 9, :],
                    in_=t["cost"].ap()[
                        128 * m + 32 * g : 128 * m + 32 * (g + 1), :
                    ],
                )

        # ---- MLP waves (SW-pipelined) + interleaved softmax/AV/proj ----
        # l_sb holds exp(logits) [r, (h, c)] in bf16 (exp applied during the
        # w2p PSUM evict, fused into the mandatory copy)
        l_sb = [
            singles.tile([128, H * C], BF16, tag=f"l{i}", name=f"l{i}")
            for i in range(2)
        ]
        pt_sb = [
            singles.tile([128, H * R], BF16, tag=f"pt{cc}", name=f"pt{cc}")
            for cc in range(2)
        ]
        ot_sb = singles.tile([128, 4 * R], BF16, tag="ot")  # [e, r]

        def expevict(m):
            # evict wave m's mixed scores from PSUM with a fused exp, then
            # scatter to the DRAM bounce; emitted one wave later so the ACT
            # queue never stalls waiting on W2(m)
            i, np_ = m // 16, m % 16
            w2p = wave_w2p[m]
            ms = mspool.tile([128, 512], BF16, tag="ms", name=f"ms{m}")
            if EXP_ON_EVICT:
                nc.scalar.activation(out=ms, in_=w2p, func=AF.Exp)
            else:
                nc.scalar.copy(out=ms, in_=w2p)
            eng = nc.sync if i == 0 else nc.gpsimd
            eng.dma_start(
                out=t["mb"].ap()[i][:, 512 * np_ : 512 * (np_ + 1)],
                in_=ms,
            )
            if np_ % 8 == 7:
                # half-rchunk gather into l_sb [r, (h, c)]
                q = (np_ // 8) % 2
                qs = slice(4096 * q, 4096 * (q + 1))
                for g in range(4):
                    src = (
                        t["mb"].ap()[i][32 * g : 32 * g + H][:, qs]
                        .rearrange("hh (nn rp c) -> hh nn rp c", nn=8, rp=2)
                        .transpose([1, 2, 0, 3])
                    )
                    dst = l_sb[i][
                        32 * g + 16 * q : 32 * g + 16 * (q + 1), :
                    ].rearrange("p (hh c) -> p hh c", hh=H)
                    eng.dma_start(out=dst, in_=src)

        def stage2(n):
            i, np_ = n // 16, n % 16
            wpa, wpd, ha, hd = wave_state[n]
            w2p = w2ps.tile([128, 512], F32, tag="w2", name=f"w2p{n}")
            wave_w2p[n] = w2p
            nc.scalar.activation(out=ha, in_=wpa, func=AF.Relu)
            nc.vector.tensor_scalar_max(out=hd, in0=wpd, scalar1=0.0)
            for g in range(4):
                nc.tensor.matmul(
                    w2p[32 * g : 32 * g + 8, :],
                    lhsT=w2_sb,
                    rhs=(ha if g < 2 else hd)[:, 512 * (g % 2) : 512 * (g % 2 + 1)],
                    start=True, stop=True,
                    tile_position=(0, 32 * g),
                )
            if n > 0:
                expevict(n - 1)

        pc_state = {}

        def phase_c_head(i, hh):
            # softmax for one head (exp already fused into the w2p evict):
            # keep-mul+rowsum_h -> recip_h -> scale_h -> 2 PE transposes ->
            # AV for the head pair on odd hh.
            if i not in pc_state:
                pc_state[i] = (
                    singles.tile([128, H], F32, tag=f"sums{i}",
                                 name=f"sums{i}"),
                    singles.tile([128, H], F32, tag=f"recips{i}",
                                 name=f"recips{i}"),
                    singles.tile([128, H * C], BF16, tag=f"pb{i}",
                                 name=f"pb{i}"),
                )
            sums, recips, pb = pc_state[i]
            hs = slice(C * hh, C * (hh + 1))
            if not EXP_ON_EVICT:
                nc.scalar.activation(out=l_sb[i][:, hs], in_=l_sb[i][:, hs],
                                     func=AF.Exp)
            nc.vector.scalar_tensor_tensor(
                out=pb[:, hs],
                in0=l_sb[i][:, hs],
                scalar=1.0,
                in1=keep_sb[:, i, :],
                op0=ALU.mult,
                op1=ALU.mult,
                accum_out=sums[:, hh : hh + 1],
            )
            nc.vector.tensor_scalar_add(
                out=sums[:, hh : hh + 1], in0=sums[:, hh : hh + 1],
                scalar1=1e-30,
            )
            nc.vector.reciprocal(
                out=recips[:, hh : hh + 1], in_=sums[:, hh : hh + 1]
            )
            nc.vector.tensor_scalar_mul(
                out=pb[:, hs], in0=pb[:, hs],
                scalar1=recips[:, hh : hh + 1],
            )
            for cc in range(2):
                tp = mmps.tile([128, 128], BF16, tag="mm",
                               name=f"tp{i}_{hh}_{cc}")
                nc.tensor.transpose(
                    tp,
                    in_=pb[:, C * hh + 128 * cc : C * hh + 128 * (cc + 1)],
                    identity=ident,
                )
                dstp = pt_sb[cc][:, R * hh + 128 * i : R * hh + 128 * (i + 1)]
                if (hh + cc) % 2 == 0:
                    nc.scalar.copy(out=dstp, in_=tp)
                else:
                    nc.vector.tensor_copy(out=dstp, in_=tp)
            if hh % 2 == 1:
                # AV for head pair (hh-1, hh), r-half i
                j = hh // 2
                ps = mmps.tile([128, 128], F32, tag="mm", name=f"av{i}_{j}")
                for s in range(2):
                    h = 2 * j + s
                    for cc in range(2):
                        nc.tensor.matmul(
                            ps[64 * s : 64 * (s + 1), :],
                            lhsT=v_sb[:, 512 * cc + 64 * h :
                                      512 * cc + 64 * (h + 1)],
                            rhs=pt_sb[cc][:, R * h + 128 * i :
                                          R * h + 128 * (i + 1)],
                            start=(cc == 0), stop=(cc == 1),
                        )
                if j % 2 == 0:
                    nc.vector.tensor_copy(
                        out=ot_sb[:, R * j + 128 * i : R * j + 128 * (i + 1)],
                        in_=ps,
                    )
                else:
                    nc.scalar.copy(
                        out=ot_sb[:, R * j + 128 * i : R * j + 128 * (i + 1)],
                        in_=ps,
                    )

        def tail(i):
            # output projection for r-half i
            ps = mmps.tile([128, 512], F32, tag="mm", name=f"yps{i}")
            for k in range(4):
                nc.tensor.matmul(
                    ps,
                    lhsT=ot_sb[:, R * k + 128 * i : R * k + 128 * (i + 1)],
                    rhs=wo_sb[:, 512 * k : 512 * (k + 1)],
                    start=(k == 0), stop=(k == 3),
                )
            y = ypool.tile([128, 512], F32, tag="y", name=f"y{i}")
            nc.scalar.copy(out=y, in_=ps)
            nc.sync.dma_start(out=t["out"].ap()[128 * i : 128 * (i + 1), :], in_=y)

        load_chunks(wo_sb, t["wo"], E, nc.gpsimd)

        wave_state = {}
        wave_w2p = {}

        def w1wave(n):
            i, np_ = n // 16, n % 16
            # split W1's PSUM output (and the h output) into independent
            # per-engine tiles -- a single shared tile serializes the ACT
            # and DVE evict halves through Tile's dependency tracking
            wpa = w1ps.tile([128, 1024], F32, tag="w1a", name=f"wpa{n}")
            wpd = w1ps.tile([128, 1024], F32, tag="w1d", name=f"wpd{n}")
            for g in range(4):
                dst = wpa if g < 2 else wpd
                nc.tensor.matmul(
                    dst[:, 512 * (g % 2) : 512 * (g % 2 + 1)],
                    lhsT=m9_sb[32 * g : 32 * g + 9, :],
                    rhs=s4[i][32 * g : 32 * g + 9, 512 * np_ : 512 * (np_ + 1)],
                    start=True, stop=True,
                    tile_position=(32 * g, 0),
                )
            wave_state[n] = (
                wpa,
                wpd,
                hpool.tile([128, 1024], BF16, tag="ha", name=f"ha{n}"),
                hpool.tile([128, 1024], BF16, tag="hd", name=f"hd{n}"),
            )

        # emission order puts stage2(n-1) (incl. its W2 matmuls) BEFORE
        # w1wave(n) so the strict PE FIFO matches data-readiness order;
        # phase_c(0) is spread one head per wave once l_sb[0] has landed
        w1wave(0)
        for n in range(1, NWAVES):
            stage2(n - 1)
            w1wave(n)
            if 19 <= n <= 26:
                phase_c_head(0, n - 19)
            if n == 27:
                tail(0)
        stage2(NWAVES - 1)
        expevict(NWAVES - 1)
        for hh in range(H):
            phase_c_head(1, hh)
        tail(1)


def _prep_inputs(row_emb, col_emb, cost_mat, attn_mask, Wq, Wk, Wv, Wo, W1,
                 W2, alpha):
    bf = ml_dtypes.bfloat16
    alpha_v = np.asarray(alpha, np.float32).reshape(-1)  # [H]
    W1 = np.asarray(W1, np.float32)
    # M9 row h (h<8): W1[2h,:]/sqrt(D); row 8: sum_h alpha_h * W1[2h+1,:]
    m9 = np.zeros((128, HID), np.float32)
    for g in range(4):
        for hh in range(H):
            m9[32 * g + hh] = W1[2 * hh] / np.sqrt(D)
        m9[32 * g + 8] = sum(alpha_v[hh] * W1[2 * hh + 1] for hh in range(H))
    shared = {
        "wq": np.asarray(Wq, np.float32).astype(bf),
        "wk": np.asarray(Wk, np.float32).astype(bf),
        "wv": np.asarray(Wv, np.float32).astype(bf),
        "wo": np.asarray(Wo, np.float32).astype(bf),
        "m9": m9.astype(bf),
        "w2": np.asarray(W2, np.float32).astype(bf),
    }
    in_maps = []
    for b in range(B):
        m = dict(shared)
        m["rembT"] = np.ascontiguousarray(
            np.asarray(row_emb[b], np.float32).T
        ).astype(bf)
        m["cembT"] = np.ascontiguousarray(
            np.asarray(col_emb[b], np.float32).T
        ).astype(bf)
        m["cost16"] = np.asarray(cost_mat[b, :, :, 0], np.float32).astype(bf)
        m["keep16"] = (~np.asarray(attn_mask[b])).astype(np.float32).astype(bf)
        in_maps.append(m)
    return in_maps


def kernel(**inputs) -> np.ndarray:
    global LAST_EXEC_NS
    if "nc" not in _CACHE:
        _CACHE["nc"] = _build()
    nc = _CACHE["nc"]
    in_maps = _prep_inputs(**inputs)
    trace = os.environ.get("KERNEL_TRACE", "0") == "1"
    res = run_bass_kernel_spmd(
        nc, in_maps, core_ids=list(range(NCORES)), trace=trace
    )
    LAST_EXEC_NS = res.exec_time_ns
    out = np.stack([np.asarray(res.results[b]["out"]) for b in range(B)])
    return out.astype(np.float32)

